# revision 1
# baseline (speedup 1.0000x reference)
"""Trainium2 Bass kernel for LGeM self-attention (b=2, t=2048, c=2048, h=16, d=128).

Sharding: 8 cores = 2 (batch, data-parallel) x 4 (head-groups of 4 heads,
tensor-parallel 'mp'). Each core computes q/k/v projections for its 4 heads,
attention, and a partial output projection (its 512 rows of Wo); the host
sums the 4 mp-partials per batch.

Math notes (matching the reference exactly):
  - rope here is q*(cos+sin) elementwise (the module's rotate_half is identity),
    folded with the 1/sqrt(t) logit scale into a precomputed per-(d,t) factor.
  - softmax is computed without max-subtraction: logits are ~N(0, 0.2^2) so
    exp never overflows; exp(x)/sum(exp(x)) == softmax(x) exactly in real math.
  - matmuls run as float32r (full-rate PE mode, fp32 storage). Walrus requires
    every fp32r-matmul input to be produced as fp32r, so all matmul-input SBUF
    tiles are allocated with dtype float32r and DRAM sources are bitcast.

Layout trick: scores are built transposed, S_T[tk, tq] = k_T_tile.T @ q_T, so
attn@v needs no transposes (stationary v[tk,d], moving exp(S_T)), and the
softmax denominator comes from a ones[128,128] stationary matmul which also
replicates the sums across all partitions (free partition-broadcast for the
reciprocal multiply). The normalized context arrives as out2_T[d, tq], which
is exactly the lhsT the output projection wants.
"""

import sys

sys.path.insert(0, "/opt/trn_rl_repo")

import math

import numpy as np

import concourse.bass as bass
import concourse.mybir as mybir
import concourse.tile as tile
from concourse import bacc, bass_utils

F32 = mybir.dt.float32
F32R = mybir.dt.float32r

HIDDEN = 2048
HEADS = 16
HEAD_DIM = 128
SEQ = 2048
BATCH = 2
N_CORES = 8
MP = 4  # tensor-parallel cores per batch
HG = HEADS // MP  # heads per core
THETA = 10000.0


def build_attention_nc(T, C, HG, D, use_mask=False):
    """Build the per-core Bass program. DG = HG*D output dims per core."""
    DG = HG * D
    CCH = C // 128  # contraction chunks for projections
    TQC = min(512, T)  # moving-dim chunk (tq)
    NTQ = T // TQC
    NTK = T // 128  # key tiles
    NQT = T // 128  # query row tiles (out proj)
    NOC = C // TQC  # out-proj column chunks

    nc = bacc.Bacc("TRN2", target_bir_lowering=False, debug=False)

    xT = nc.dram_tensor("xT", [C, T], F32, kind="ExternalInput").ap()
    wq = nc.dram_tensor("wq", [C, DG], F32, kind="ExternalInput").ap()
    wk = nc.dram_tensor("wk", [C, DG], F32, kind="ExternalInput").ap()
    wv = nc.dram_tensor("wv", [C, DG], F32, kind="ExternalInput").ap()
    wo = nc.dram_tensor("wo", [DG, C], F32, kind="ExternalInput").ap()
    cfq = nc.dram_tensor("cfq", [D, T], F32, kind="ExternalInput").ap()
    cfk = nc.dram_tensor("cfk", [D, T], F32, kind="ExternalInput").ap()
    if use_mask:
        maskT = nc.dram_tensor("maskT", [T, T], F32, kind="ExternalInput").ap()
    out = nc.dram_tensor("out", [T, C], F32, kind="ExternalOutput").ap()

    with tile.TileContext(nc) as tc:
        with tc.tile_pool(name="scratch", bufs=1, space="DRAM") as dpool:
            qT_s = dpool.tile([DG, T], F32R)
            kT_s = dpool.tile([DG, T], F32R)
            v_s = dpool.tile([T, DG], F32R)

            # ---------------- Phase A: projections ----------------
            with tc.tile_pool(name="xp", bufs=1) as xpool:
                xT_sb = xpool.tile([128, CCH * T], F32R)
                for cc in range(CCH):
                    nc.sync.dma_start(
                        xT_sb[:, cc * T : (cc + 1) * T],
                        xT[cc * 128 : (cc + 1) * 128, :].bitcast(F32R),
                    )

                # v = x @ Wv  ->  v[tk, dout], stationary xT tiles, moving Wv
                with (
                    tc.tile_pool(name="wvp", bufs=1) as wvpool,
                    tc.tile_pool(name="vst", bufs=3) as vstpool,
                    tc.tile_pool(name="vps", bufs=4, space="PSUM") as vps,
                ):
                    wv_sb = wvpool.tile([128, CCH * DG], F32R)
                    for cc in range(CCH):
                        nc.sync.dma_start(
                            wv_sb[:, cc * DG : (cc + 1) * DG],
                            wv[cc * 128 : (cc + 1) * 128, :].bitcast(F32R),
                        )
                    for tk in range(NTK):
                        pv = vps.tile([128, DG], F32)
                        for cc in range(CCH):
                            nc.tensor.matmul(
                                pv[:],
                                xT_sb[:, cc * T + tk * 128 : cc * T + (tk + 1) * 128],
                                wv_sb[:, cc * DG : (cc + 1) * DG],
                                start=(cc == 0),
                                stop=(cc == CCH - 1),
                            )
                        vt = vstpool.tile([128, DG], F32R)
                        nc.vector.tensor_copy(vt[:], pv[:])
                        nc.sync.dma_start(v_s[tk * 128 : (tk + 1) * 128, :], vt[:])

                # q_T = (Wq_h).T @ x_T (then * cfq), k_T likewise (* cfk)
                with (
                    tc.tile_pool(name="cf", bufs=1) as cfpool,
                    tc.tile_pool(name="wqk", bufs=2) as wpool,
                    tc.tile_pool(name="qkst", bufs=2) as stpool,
                    tc.tile_pool(name="qkps", bufs=4, space="PSUM") as qkps,
                ):
                    cfq_sb = cfpool.tile([128, T], F32, tag="cfq")
                    cfk_sb = cfpool.tile([128, T], F32, tag="cfk")
                    nc.sync.dma_start(cfq_sb[:D, :], cfq)
                    nc.sync.dma_start(cfk_sb[:D, :], cfk)
                    for h in range(HG):
                        for w_in, cf_sb, dst in (
                            (wq, cfq_sb, qT_s),
                            (wk, cfk_sb, kT_s),
                        ):
                            w_sb = wpool.tile([128, CCH * D], F32R, tag="w")
                            for cc in range(CCH):
                                nc.sync.dma_start(
                                    w_sb[:, cc * D : (cc + 1) * D],
                                    w_in[
                                        cc * 128 : (cc + 1) * 128,
                                        h * D : (h + 1) * D,
                                    ].bitcast(F32R),
                                )
                            stage = stpool.tile([128, T], F32R, tag="st")
                            for tq in range(NTQ):
                                pm = qkps.tile([128, TQC], F32)
                                for cc in range(CCH):
                                    nc.tensor.matmul(
                                        pm[:],
                                        w_sb[:, cc * D : (cc + 1) * D],
                                        xT_sb[
                                            :,
                                            cc * T + tq * TQC : cc * T + (tq + 1) * TQC,
                                        ],
                                        start=(cc == 0),
                                        stop=(cc == CCH - 1),
                                    )
                                nc.vector.tensor_mul(
                                    stage[:D, tq * TQC : (tq + 1) * TQC],
                                    pm[:D, :],
                                    cf_sb[:D, tq * TQC : (tq + 1) * TQC],
                                )
                            nc.sync.dma_start(dst[h * D : (h + 1) * D, :], stage[:D, :])

            # ---------------- Phase B: attention ----------------
            with tc.tile_pool(name="o2", bufs=1) as o2pool:
                out2_sb = o2pool.tile([128, HG * T], F32R)
                with (
                    tc.tile_pool(name="hp", bufs=2) as hpool,
                    tc.tile_pool(name="cst", bufs=1) as cstpool,
                    tc.tile_pool(name="ep", bufs=6) as epool,
                    tc.tile_pool(name="mp", bufs=4) as mpool,
                    tc.tile_pool(name="rp", bufs=2) as rpool,
                    tc.tile_pool(name="scps", bufs=4, space="PSUM") as scps,
                    tc.tile_pool(name="o2ps", bufs=2, space="PSUM") as o2ps,
                    tc.tile_pool(name="sps", bufs=2, space="PSUM") as sps,
                ):
                    ones_f = cstpool.tile([128, 128], F32)
                    nc.vector.memset(ones_f[:], 1.0)
                    ones = cstpool.tile([128, 128], F32R)
                    nc.vector.tensor_copy(ones[:], ones_f[:])
                    for h in range(HG):
                        qT_sb = hpool.tile([128, T], F32R, tag="qT")
                        kT_sb = hpool.tile([128, T], F32R, tag="kT")
                        v_sb = hpool.tile([128, NTK * D], F32R, tag="v")
                        nc.sync.dma_start(qT_sb[:D, :], qT_s[h * D : (h + 1) * D, :])
                        nc.sync.dma_start(kT_sb[:D, :], kT_s[h * D : (h + 1) * D, :])
                        for i in range(NTK):
                            nc.sync.dma_start(
                                v_sb[:, i * D : (i + 1) * D],
                                v_s[i * 128 : (i + 1) * 128, h * D : (h + 1) * D],
                            )
                        for tq in range(NTQ):
                            o2p = o2ps.tile([128, TQC], F32)
                            sp = sps.tile([128, TQC], F32)
                            # Software-pipelined: the accumulation matmuls for
                            # tile tk are emitted after the scores matmul of
                            # tk+1, so PE has independent work while ACT exps.
                            pending = None
                            for tk in range(NTK):
                                scp = scps.tile([128, TQC], F32)
                                nc.tensor.matmul(
                                    scp[:],
                                    kT_sb[:D, tk * 128 : (tk + 1) * 128],
                                    qT_sb[:D, tq * TQC : (tq + 1) * TQC],
                                    start=True,
                                    stop=True,
                                )
                                et = epool.tile([128, TQC], F32R, tag="et")
                                if use_mask:
                                    mt = mpool.tile([128, TQC], F32, tag="mt")
                                    nc.sync.dma_start(
                                        mt[:],
                                        maskT[
                                            tk * 128 : (tk + 1) * 128,
                                            tq * TQC : (tq + 1) * TQC,
                                        ],
                                    )
                                    ma = mpool.tile([128, TQC], F32, tag="ma")
                                    nc.vector.tensor_add(ma[:], scp[:], mt[:])
                                    nc.scalar.activation(
                                        et[:],
                                        ma[:],
                                        mybir.ActivationFunctionType.Exp,
                                    )
                                else:
                                    nc.scalar.activation(
                                        et[:],
                                        scp[:],
                                        mybir.ActivationFunctionType.Exp,
                                    )
                                if pending is not None:
                                    p_et, p_tk = pending
                                    nc.tensor.matmul(
                                        o2p[:],
                                        v_sb[:, p_tk * D : (p_tk + 1) * D],
                                        p_et[:],
                                        start=(p_tk == 0),
                                        stop=False,
                                    )
                                    nc.tensor.matmul(
                                        sp[:],
                                        ones[:],
                                        p_et[:],
                                        start=(p_tk == 0),
                                        stop=False,
                                    )
                                pending = (et, tk)
                            p_et, p_tk = pending
                            nc.tensor.matmul(
                                o2p[:],
                                v_sb[:, p_tk * D : (p_tk + 1) * D],
                                p_et[:],
                                start=False,
                                stop=True,
                            )
                            nc.tensor.matmul(
                                sp[:],
                                ones[:],
                                p_et[:],
                                start=False,
                                stop=True,
                            )
                            rt = rpool.tile([128, TQC], F32)
                            nc.vector.reciprocal(rt[:], sp[:])
                            nc.vector.tensor_mul(
                                out2_sb[:D, h * T + tq * TQC : h * T + (tq + 1) * TQC],
                                o2p[:D, :],
                                rt[:D, :],
                            )

                # -------- output projection (partial over this core's heads) ----
                with (
                    tc.tile_pool(name="wop", bufs=1) as wopool,
                    tc.tile_pool(name="fst", bufs=4) as fpool,
                    tc.tile_pool(name="fps", bufs=4, space="PSUM") as fps,
                ):
                    wo_sb = wopool.tile([128, HG * C], F32R)
                    for h in range(HG):
                        nc.sync.dma_start(
                            wo_sb[:D, h * C : (h + 1) * C],
                            wo[h * D : (h + 1) * D, :].bitcast(F32R),
                        )
                    for qt in range(NQT):
                        for oc in range(NOC):
                            fp = fps.tile([128, TQC], F32)
                            for h in range(HG):
                                nc.tensor.matmul(
                                    fp[:],
                                    out2_sb[
                                        :D, h * T + qt * 128 : h * T + (qt + 1) * 128
                                    ],
                                    wo_sb[
                                        :D, h * C + oc * TQC : h * C + (oc + 1) * TQC
                                    ],
                                    start=(h == 0),
                                    stop=(h == HG - 1),
                                )
                            ft = fpool.tile([128, TQC], F32, tag="ft")
                            nc.vector.tensor_copy(ft[:], fp[:])
                            nc.sync.dma_start(
                                out[
                                    qt * 128 : (qt + 1) * 128,
                                    oc * TQC : (oc + 1) * TQC,
                                ],
                                ft[:],
                            )

    nc.compile()
    return nc


def compute_cfacs(T, D, theta=THETA):
    """cfq = (cos+sin).T / sqrt(T)  [D, T];  cfk = (cos+sin).T  [D, T]."""
    freq = 1.0 / theta ** (np.arange(0, D, 2, dtype=np.float64) / D)
    t = np.arange(T, dtype=np.float64)
    m = np.einsum("i,j->ij", t, freq)  # [T, D/2]
    m = np.concatenate([m, m], axis=-1)  # [T, D]
    cfac = (np.cos(m) + np.sin(m)).astype(np.float32)  # [T, D]
    cfk = np.ascontiguousarray(cfac.T)  # [D, T]
    cfq = np.ascontiguousarray(cfac.T / np.float32(math.sqrt(T))).astype(np.float32)
    return cfq, cfk


_NC_CACHE = {}


def _get_nc(use_mask):
    key = bool(use_mask)
    if key not in _NC_CACHE:
        _NC_CACHE[key] = build_attention_nc(SEQ, HIDDEN, HG, HEAD_DIM, use_mask=key)
    return _NC_CACHE[key]


def kernel(input_ids, attention_mask, Wq, Wk, Wv, Wo):
    input_ids = np.asarray(input_ids, dtype=np.float32)
    attention_mask = np.asarray(attention_mask, dtype=np.float32)
    Wq = np.asarray(Wq, dtype=np.float32)
    Wk = np.asarray(Wk, dtype=np.float32)
    Wv = np.asarray(Wv, dtype=np.float32)
    Wo = np.asarray(Wo, dtype=np.float32)

    b, t, c = input_ids.shape
    assert (b, t, c) == (BATCH, SEQ, HIDDEN)
    DG = HG * HEAD_DIM

    use_mask = bool(np.any(attention_mask))
    nc = _get_nc(use_mask)

    cfq, cfk = compute_cfacs(SEQ, HEAD_DIM)

    in_maps = []
    for core in range(N_CORES):
        bi, g = divmod(core, MP)
        m = {
            "xT": np.ascontiguousarray(input_ids[bi].T),
            "wq": np.ascontiguousarray(Wq[:, g * DG : (g + 1) * DG]),
            "wk": np.ascontiguousarray(Wk[:, g * DG : (g + 1) * DG]),
            "wv": np.ascontiguousarray(Wv[:, g * DG : (g + 1) * DG]),
            "wo": np.ascontiguousarray(Wo[g * DG : (g + 1) * DG, :]),
            "cfq": cfq,
            "cfk": cfk,
        }
        if use_mask:
            m["maskT"] = np.ascontiguousarray(attention_mask[bi, 0].T)
        in_maps.append(m)

    res = bass_utils.run_bass_kernel_spmd(nc, in_maps, core_ids=list(range(N_CORES)))

    out = np.zeros((BATCH, SEQ, HIDDEN), dtype=np.float32)
    for bi in range(BATCH):
        acc = res.results[bi * MP]["out"].astype(np.float32)
        for g in range(1, MP):
            acc = acc + res.results[bi * MP + g]["out"]
        out[bi] = acc
    return out



# revision 2
# speedup vs baseline: 1.0387x; 1.0387x over previous
"""Trainium2 Bass kernel for LGeM self-attention (b=2, t=2048, c=2048, h=16, d=128).

Sharding: 8 cores = 2 (batch, data-parallel) x 4 (head-groups of 4 heads,
tensor-parallel 'mp'). Each core computes q/k/v projections for its 4 heads,
attention, and a partial output projection (its 512 rows of Wo); the host
sums the 4 mp-partials per batch.

Math notes (matching the reference exactly):
  - rope here is q*(cos+sin) elementwise (the module's rotate_half is identity),
    folded with the 1/sqrt(t) logit scale into a precomputed per-(d,t) factor.
  - softmax is computed without max-subtraction: logits are ~N(0, 0.2^2) so
    exp never overflows; exp(x)/sum(exp(x)) == softmax(x) exactly in real math.
  - matmuls run as float32r (full-rate PE mode, fp32 storage). Walrus requires
    every fp32r-matmul input to be produced as fp32r, so all matmul-input SBUF
    tiles are allocated with dtype float32r and DRAM sources are bitcast.

Layout trick: scores are built transposed, S_T[tk, tq] = k_T_tile.T @ q_T, so
attn@v needs no transposes (stationary v[tk,d], moving exp(S_T)), and the
softmax denominator comes from a ones[128,128] stationary matmul which also
replicates the sums across all partitions (free partition-broadcast for the
reciprocal multiply). The normalized context arrives as out2_T[d, tq], which
is exactly the lhsT the output projection wants.
"""

import sys

sys.path.insert(0, "/opt/trn_rl_repo")

import math

import numpy as np

import concourse.bass as bass
import concourse.mybir as mybir
import concourse.tile as tile
from concourse import bacc, bass_utils

F32 = mybir.dt.float32
F32R = mybir.dt.float32r

HIDDEN = 2048
HEADS = 16
HEAD_DIM = 128
SEQ = 2048
BATCH = 2
N_CORES = 8
MP = 4  # tensor-parallel cores per batch
HG = HEADS // MP  # heads per core
THETA = 10000.0


def build_attention_nc(T, C, HG, D, use_mask=False):
    """Build the per-core Bass program. DG = HG*D output dims per core."""
    DG = HG * D
    CCH = C // 128  # contraction chunks for projections
    TQC = min(512, T)  # moving-dim chunk (tq)
    NTQ = T // TQC
    NTK = T // 128  # key tiles
    NQT = T // 128  # query row tiles (out proj)
    NOC = C // TQC  # out-proj column chunks

    nc = bacc.Bacc("TRN2", target_bir_lowering=False, debug=False)

    xT = nc.dram_tensor("xT", [C, T], F32, kind="ExternalInput").ap()
    wq = nc.dram_tensor("wq", [C, DG], F32, kind="ExternalInput").ap()
    wk = nc.dram_tensor("wk", [C, DG], F32, kind="ExternalInput").ap()
    wv = nc.dram_tensor("wv", [C, DG], F32, kind="ExternalInput").ap()
    wo = nc.dram_tensor("wo", [DG, C], F32, kind="ExternalInput").ap()
    cfq = nc.dram_tensor("cfq", [D, T], F32, kind="ExternalInput").ap()
    cfk = nc.dram_tensor("cfk", [D, T], F32, kind="ExternalInput").ap()
    if use_mask:
        maskT = nc.dram_tensor("maskT", [T, T], F32, kind="ExternalInput").ap()
    out = nc.dram_tensor("out", [T, C], F32, kind="ExternalOutput").ap()

    with tile.TileContext(nc) as tc:
        with tc.tile_pool(name="scratch", bufs=1, space="DRAM") as dpool:
            qT_s = dpool.tile([DG, T], F32R)
            kT_s = dpool.tile([DG, T], F32R)
            v_s = dpool.tile([T, DG], F32R)

            # ---------------- Phase A: projections ----------------
            with tc.tile_pool(name="xp", bufs=1) as xpool:
                xT_sb = xpool.tile([128, CCH * T], F32R)
                for cc in range(CCH):
                    nc.sync.dma_start(
                        xT_sb[:, cc * T : (cc + 1) * T],
                        xT[cc * 128 : (cc + 1) * 128, :].bitcast(F32R),
                    )

                # v = x @ Wv  ->  v[tk, dout], stationary xT tiles, moving Wv
                with (
                    tc.tile_pool(name="wvp", bufs=1) as wvpool,
                    tc.tile_pool(name="vst", bufs=3) as vstpool,
                    tc.tile_pool(name="vps", bufs=4, space="PSUM") as vps,
                ):
                    wv_sb = wvpool.tile([128, CCH * DG], F32R)
                    for cc in range(CCH):
                        nc.sync.dma_start(
                            wv_sb[:, cc * DG : (cc + 1) * DG],
                            wv[cc * 128 : (cc + 1) * 128, :].bitcast(F32R),
                        )
                    for tk in range(NTK):
                        pv = vps.tile([128, DG], F32)
                        for cc in range(CCH):
                            nc.tensor.matmul(
                                pv[:],
                                xT_sb[:, cc * T + tk * 128 : cc * T + (tk + 1) * 128],
                                wv_sb[:, cc * DG : (cc + 1) * DG],
                                start=(cc == 0),
                                stop=(cc == CCH - 1),
                            )
                        vt = vstpool.tile([128, DG], F32R)
                        nc.vector.tensor_copy(vt[:], pv[:])
                        nc.sync.dma_start(v_s[tk * 128 : (tk + 1) * 128, :], vt[:])

                # q_T = (Wq_h).T @ x_T (then * cfq), k_T likewise (* cfk)
                with (
                    tc.tile_pool(name="cf", bufs=1) as cfpool,
                    tc.tile_pool(name="wqk", bufs=2) as wpool,
                    tc.tile_pool(name="qkst", bufs=2) as stpool,
                    tc.tile_pool(name="qkps", bufs=4, space="PSUM") as qkps,
                ):
                    cfq_sb = cfpool.tile([128, T], F32, tag="cfq")
                    cfk_sb = cfpool.tile([128, T], F32, tag="cfk")
                    nc.sync.dma_start(cfq_sb[:D, :], cfq)
                    nc.sync.dma_start(cfk_sb[:D, :], cfk)
                    for h in range(HG):
                        for w_in, cf_sb, dst in (
                            (wq, cfq_sb, qT_s),
                            (wk, cfk_sb, kT_s),
                        ):
                            w_sb = wpool.tile([128, CCH * D], F32R, tag="w")
                            for cc in range(CCH):
                                nc.sync.dma_start(
                                    w_sb[:, cc * D : (cc + 1) * D],
                                    w_in[
                                        cc * 128 : (cc + 1) * 128,
                                        h * D : (h + 1) * D,
                                    ].bitcast(F32R),
                                )
                            stage = stpool.tile([128, T], F32R, tag="st")
                            for tq in range(NTQ):
                                pm = qkps.tile([128, TQC], F32)
                                for cc in range(CCH):
                                    nc.tensor.matmul(
                                        pm[:],
                                        w_sb[:, cc * D : (cc + 1) * D],
                                        xT_sb[
                                            :,
                                            cc * T + tq * TQC : cc * T + (tq + 1) * TQC,
                                        ],
                                        start=(cc == 0),
                                        stop=(cc == CCH - 1),
                                    )
                                nc.vector.tensor_mul(
                                    stage[:D, tq * TQC : (tq + 1) * TQC],
                                    pm[:D, :],
                                    cf_sb[:D, tq * TQC : (tq + 1) * TQC],
                                )
                            nc.sync.dma_start(dst[h * D : (h + 1) * D, :], stage[:D, :])

            # ---------------- Phase B: attention ----------------
            with tc.tile_pool(name="o2", bufs=1) as o2pool:
                out2_sb = o2pool.tile([128, HG * T], F32R)
                with (
                    tc.tile_pool(name="hp", bufs=2) as hpool,
                    tc.tile_pool(name="cst", bufs=1) as cstpool,
                    tc.tile_pool(name="ep", bufs=6) as epool,
                    tc.tile_pool(name="mp", bufs=4) as mpool,
                    tc.tile_pool(name="rp", bufs=2) as rpool,
                    tc.tile_pool(name="scps", bufs=4, space="PSUM") as scps,
                    tc.tile_pool(name="o2ps", bufs=2, space="PSUM") as o2ps,
                    tc.tile_pool(name="sps", bufs=2, space="PSUM") as sps,
                ):
                    ones_f = cstpool.tile([128, 128], F32)
                    nc.vector.memset(ones_f[:], 1.0)
                    ones = cstpool.tile([128, 128], F32R)
                    nc.vector.tensor_copy(ones[:], ones_f[:])
                    for h in range(HG):
                        qT_sb = hpool.tile([128, T], F32R, tag="qT")
                        kT_sb = hpool.tile([128, T], F32R, tag="kT")
                        v_sb = hpool.tile([128, NTK * D], F32R, tag="v")
                        nc.sync.dma_start(qT_sb[:D, :], qT_s[h * D : (h + 1) * D, :])
                        nc.sync.dma_start(kT_sb[:D, :], kT_s[h * D : (h + 1) * D, :])
                        for i in range(NTK):
                            nc.sync.dma_start(
                                v_sb[:, i * D : (i + 1) * D],
                                v_s[i * 128 : (i + 1) * 128, h * D : (h + 1) * D],
                            )
                        for tq in range(NTQ):
                            o2p = o2ps.tile([128, TQC], F32)
                            sp = sps.tile([128, TQC], F32)
                            # Software-pipelined: the accumulation matmuls for
                            # tile tk are emitted after the scores matmul of
                            # tk+1, so PE has independent work while ACT exps.
                            pending = None
                            for tk in range(NTK):
                                scp = scps.tile([128, TQC], F32)
                                nc.tensor.matmul(
                                    scp[:],
                                    kT_sb[:D, tk * 128 : (tk + 1) * 128],
                                    qT_sb[:D, tq * TQC : (tq + 1) * TQC],
                                    start=True,
                                    stop=True,
                                )
                                et = epool.tile([128, TQC], F32R, tag="et")
                                if use_mask:
                                    mt = mpool.tile([128, TQC], F32, tag="mt")
                                    nc.sync.dma_start(
                                        mt[:],
                                        maskT[
                                            tk * 128 : (tk + 1) * 128,
                                            tq * TQC : (tq + 1) * TQC,
                                        ],
                                    )
                                    ma = mpool.tile([128, TQC], F32, tag="ma")
                                    nc.vector.tensor_add(ma[:], scp[:], mt[:])
                                    nc.scalar.activation(
                                        et[:],
                                        ma[:],
                                        mybir.ActivationFunctionType.Exp,
                                    )
                                else:
                                    nc.scalar.activation(
                                        et[:],
                                        scp[:],
                                        mybir.ActivationFunctionType.Exp,
                                    )
                                if pending is not None:
                                    p_et, p_tk = pending
                                    nc.tensor.matmul(
                                        o2p[:],
                                        v_sb[:, p_tk * D : (p_tk + 1) * D],
                                        p_et[:],
                                        start=(p_tk == 0),
                                        stop=False,
                                    )
                                    nc.tensor.matmul(
                                        sp[:],
                                        ones[:],
                                        p_et[:],
                                        start=(p_tk == 0),
                                        stop=False,
                                    )
                                pending = (et, tk)
                            p_et, p_tk = pending
                            nc.tensor.matmul(
                                o2p[:],
                                v_sb[:, p_tk * D : (p_tk + 1) * D],
                                p_et[:],
                                start=False,
                                stop=True,
                            )
                            nc.tensor.matmul(
                                sp[:],
                                ones[:],
                                p_et[:],
                                start=False,
                                stop=True,
                            )
                            rt = rpool.tile([128, TQC], F32)
                            nc.vector.reciprocal(rt[:], sp[:])
                            nc.vector.tensor_mul(
                                out2_sb[:D, h * T + tq * TQC : h * T + (tq + 1) * TQC],
                                o2p[:D, :],
                                rt[:D, :],
                            )

                # -------- output projection (partial over this core's heads) ----
                with (
                    tc.tile_pool(name="wop", bufs=1) as wopool,
                    tc.tile_pool(name="fst", bufs=4) as fpool,
                    tc.tile_pool(name="fps", bufs=4, space="PSUM") as fps,
                ):
                    wo_sb = wopool.tile([128, HG * C], F32R)
                    for h in range(HG):
                        nc.sync.dma_start(
                            wo_sb[:D, h * C : (h + 1) * C],
                            wo[h * D : (h + 1) * D, :].bitcast(F32R),
                        )
                    for qt in range(NQT):
                        for oc in range(NOC):
                            fp = fps.tile([128, TQC], F32)
                            for h in range(HG):
                                nc.tensor.matmul(
                                    fp[:],
                                    out2_sb[
                                        :D, h * T + qt * 128 : h * T + (qt + 1) * 128
                                    ],
                                    wo_sb[
                                        :D, h * C + oc * TQC : h * C + (oc + 1) * TQC
                                    ],
                                    start=(h == 0),
                                    stop=(h == HG - 1),
                                )
                            ft = fpool.tile([128, TQC], F32, tag="ft")
                            nc.vector.tensor_copy(ft[:], fp[:])
                            nc.sync.dma_start(
                                out[
                                    qt * 128 : (qt + 1) * 128,
                                    oc * TQC : (oc + 1) * TQC,
                                ],
                                ft[:],
                            )

    nc.compile()
    return nc


def compute_cfacs(T, D, theta=THETA):
    """cfq = (cos+sin).T / sqrt(T)  [D, T];  cfk = (cos+sin).T  [D, T]."""
    freq = 1.0 / theta ** (np.arange(0, D, 2, dtype=np.float64) / D)
    t = np.arange(T, dtype=np.float64)
    m = np.einsum("i,j->ij", t, freq)  # [T, D/2]
    m = np.concatenate([m, m], axis=-1)  # [T, D]
    cfac = (np.cos(m) + np.sin(m)).astype(np.float32)  # [T, D]
    cfk = np.ascontiguousarray(cfac.T)  # [D, T]
    cfq = np.ascontiguousarray(cfac.T / np.float32(math.sqrt(T))).astype(np.float32)
    return cfq, cfk


_NC_CACHE = {}


def _get_nc(use_mask):
    key = bool(use_mask)
    if key not in _NC_CACHE:
        _NC_CACHE[key] = build_attention_nc(SEQ, HIDDEN, HG, HEAD_DIM, use_mask=key)
    return _NC_CACHE[key]


def prepare_for_bench(inputs):
    """Return (nc, in_maps) for external timing harnesses."""
    input_ids = np.asarray(inputs["input_ids"], dtype=np.float32)
    Wq = np.asarray(inputs["Wq"], dtype=np.float32)
    Wk = np.asarray(inputs["Wk"], dtype=np.float32)
    Wv = np.asarray(inputs["Wv"], dtype=np.float32)
    Wo = np.asarray(inputs["Wo"], dtype=np.float32)
    DG = HG * HEAD_DIM
    nc = _get_nc(False)
    cfq, cfk = compute_cfacs(SEQ, HEAD_DIM)
    in_maps = []
    for core in range(N_CORES):
        bi, g = divmod(core, MP)
        in_maps.append(
            {
                "xT": np.ascontiguousarray(input_ids[bi].T),
                "wq": np.ascontiguousarray(Wq[:, g * DG : (g + 1) * DG]),
                "wk": np.ascontiguousarray(Wk[:, g * DG : (g + 1) * DG]),
                "wv": np.ascontiguousarray(Wv[:, g * DG : (g + 1) * DG]),
                "wo": np.ascontiguousarray(Wo[g * DG : (g + 1) * DG, :]),
                "cfq": cfq,
                "cfk": cfk,
            }
        )
    return nc, in_maps


def kernel(input_ids, attention_mask, Wq, Wk, Wv, Wo):
    input_ids = np.asarray(input_ids, dtype=np.float32)
    attention_mask = np.asarray(attention_mask, dtype=np.float32)
    Wq = np.asarray(Wq, dtype=np.float32)
    Wk = np.asarray(Wk, dtype=np.float32)
    Wv = np.asarray(Wv, dtype=np.float32)
    Wo = np.asarray(Wo, dtype=np.float32)

    b, t, c = input_ids.shape
    assert (b, t, c) == (BATCH, SEQ, HIDDEN)
    DG = HG * HEAD_DIM

    use_mask = bool(np.any(attention_mask))
    nc = _get_nc(use_mask)

    cfq, cfk = compute_cfacs(SEQ, HEAD_DIM)

    in_maps = []
    for core in range(N_CORES):
        bi, g = divmod(core, MP)
        m = {
            "xT": np.ascontiguousarray(input_ids[bi].T),
            "wq": np.ascontiguousarray(Wq[:, g * DG : (g + 1) * DG]),
            "wk": np.ascontiguousarray(Wk[:, g * DG : (g + 1) * DG]),
            "wv": np.ascontiguousarray(Wv[:, g * DG : (g + 1) * DG]),
            "wo": np.ascontiguousarray(Wo[g * DG : (g + 1) * DG, :]),
            "cfq": cfq,
            "cfk": cfk,
        }
        if use_mask:
            m["maskT"] = np.ascontiguousarray(attention_mask[bi, 0].T)
        in_maps.append(m)

    res = bass_utils.run_bass_kernel_spmd(nc, in_maps, core_ids=list(range(N_CORES)))

    out = np.zeros((BATCH, SEQ, HIDDEN), dtype=np.float32)
    for bi in range(BATCH):
        acc = res.results[bi * MP]["out"].astype(np.float32)
        for g in range(1, MP):
            acc = acc + res.results[bi * MP + g]["out"]
        out[bi] = acc
    return out



# revision 4
# speedup vs baseline: 1.1139x; 1.0724x over previous
"""v12: v9 + two-sweep v-projection: sweep 1 accumulates 8 tk tiles across
all 8 PSUM banks chunk-by-chunk as x arrives (PE saturated during the x
load); sweep 2 finishes tk 8-15 once x is resident.

v9: v8 but per-head attention reads are normal phase-B loads on the idle
Pool queue (v8 emitted them in phase A with a single-buffered pool that
parked the SP queue head on a long semaphore wait - and hit
NRT_EXEC_UNIT_UNRECOVERABLE on hardware).

Original: Trainium2 Bass kernel for LGeM self-attention (b=2, t=2048, c=2048, h=16, d=128).

v8: fp32r matmuls (self-loading weights; bf16 matmuls pay a ~550ns/mm
ldweights penalty on real TRN2; walrus rejects mixed-dtype matmuls and
ldw-opt). All host->device tensors ship bf16; upcasts to fp32r run on DVE
(8-deep exec queue) and ACT, which are idle during the projections.

Schedule/queue design (from timeline-sim gap analysis):
  - pools open before any emission so no engine's first DMA waits on pool
    allocation barriers; wv chunk loads (Pool/SWDGE queue) are emitted
    before the x loads so the first v-proj matmul starts ~3us in,
  - x chunks upcast through per-chunk fp32r tiles (dependency granularity
    = one chunk, not the whole 128KB tile),
  - wq/wk slices load on the ACT queue (idle after the x odd-chunk loads);
    Pool/SWDGE is slow (~1us/DMA engine time) so it only carries wv/wo,
  - qT and v round-trip a bf16 DRAM scratch on the SP queue; the per-head
    attention reads are emitted inside phase A right after their head's
    scratch writes, so they land long before attention needs them,
  - kT stays SBUF-resident bf16; cf factors ship bf16 and feed mixed-dtype
    DVE multiplies (f32 PSUM x bf16 -> bf16),
  - output is written bf16; host sums the mp-partials in fp32.

Numerics: the validated "bf16 everywhere" dataflow (5e-3 max rel err vs fp32
reference; tolerance 2e-2) with fp32r accumulation between stages.

Sharding: 8 cores = 2 (batch) x 4 (head-groups of 4 heads). Per core: q/k/v
projections for its 4 heads, attention, partial output projection (its 512
rows of Wo); host sums the 4 mp-partials per batch.

Math notes (matching the reference exactly):
  - rope is q*(cos+sin) elementwise (the module's rotate_half is identity),
    with the 1/sqrt(t) logit scale folded into cfq.
  - softmax without max-subtraction: logits ~N(0,0.2^2), exp cannot overflow.
  - scores are built transposed, S_T[tk,tq] = kT_tile.T @ qT, so attn@v needs
    no transposes; the softmax denominator comes from a ones[128,128]
    stationary matmul (free partition-broadcast for the reciprocal multiply).
"""

import sys

sys.path.insert(0, "/opt/trn_rl_repo")

import math

import numpy as np
import ml_dtypes

import concourse.bass as bass
import concourse.mybir as mybir
import concourse.tile as tile
from concourse import bacc, bass_utils

F32 = mybir.dt.float32
F32R = mybir.dt.float32r
BF16 = mybir.dt.bfloat16
NP_BF16 = ml_dtypes.bfloat16
Copy = mybir.ActivationFunctionType.Copy
Exp = mybir.ActivationFunctionType.Exp

HIDDEN = 2048
HEADS = 16
HEAD_DIM = 128
SEQ = 2048
BATCH = 2
N_CORES = 8
MP = 4
HG = HEADS // MP
THETA = 10000.0


def build_attention_nc(T, C, HG, D, use_mask=False):
    DG = HG * D  # 512
    CCH = C // 128  # 16
    TQC = min(512, T)
    NTQ = T // TQC  # 4
    NTK = T // 128  # 16
    NQT = T // 128  # 16
    NOC = C // TQC  # 4

    nc = bacc.Bacc("TRN2", target_bir_lowering=False, debug=False)

    xT = nc.dram_tensor("xT", [C, T], BF16, kind="ExternalInput").ap()
    wq = nc.dram_tensor("wq", [C, DG], BF16, kind="ExternalInput").ap()
    wk = nc.dram_tensor("wk", [C, DG], BF16, kind="ExternalInput").ap()
    wv = nc.dram_tensor("wv", [C, DG], BF16, kind="ExternalInput").ap()
    wo = nc.dram_tensor("wo", [DG, C], BF16, kind="ExternalInput").ap()
    cfq = nc.dram_tensor("cfq", [D, T], BF16, kind="ExternalInput").ap()
    cfk = nc.dram_tensor("cfk", [D, T], BF16, kind="ExternalInput").ap()
    if use_mask:
        maskT = nc.dram_tensor("maskT", [T, T], F32, kind="ExternalInput").ap()
    out = nc.dram_tensor("out", [T, C], BF16, kind="ExternalOutput").ap()

    with tile.TileContext(nc) as tc:
        with (
            tc.tile_pool(name="res", bufs=1) as respool,
            tc.tile_pool(name="scratch", bufs=1, space="DRAM") as dpool,
        ):
            kT_bf = respool.tile([128, HG, T], BF16, tag="kTb")  # [d, h, t]
            qT_s = dpool.tile([DG, T], BF16, tag="qTs")
            v_s = dpool.tile([T, DG], BF16, tag="vs")

            # ---------------- Phase A ----------------
            with tc.tile_pool(name="xp", bufs=1) as xpool:
                xT_c = [
                    xpool.tile([128, T], F32R, name=f"xc{cc}", tag=f"xc{cc}")
                    for cc in range(CCH)
                ]
                with tc.tile_pool(name="cf", bufs=1) as cfpool:
                    cfq_sb = cfpool.tile([128, T], BF16, tag="cfq")
                    cfk_sb = cfpool.tile([128, T], BF16, tag="cfk")

                    with (
                        tc.tile_pool(name="xbf", bufs=3) as xbfpool,
                        tc.tile_pool(name="wvbf", bufs=1) as wvbfpool,
                        tc.tile_pool(name="wvf", bufs=1) as wvfpool,
                        tc.tile_pool(name="vst", bufs=2) as vstpool,
                        tc.tile_pool(name="vps", bufs=1, space="PSUM") as vps,
                    ):
                        # wv first on the Pool queue: first chunks land ~2us
                        wv_c = [
                            wvfpool.tile(
                                [128, DG], F32R, name=f"wvc{cc}", tag=f"wvc{cc}"
                            )
                            for cc in range(CCH)
                        ]
                        for cc in range(CCH):
                            wvb = wvbfpool.tile([128, DG], BF16, tag="wvb")
                            nc.gpsimd.dma_start(
                                wvb[:], wv[cc * 128 : (cc + 1) * 128, :]
                            )
                            if cc % 2 == 0:
                                nc.vector.tensor_copy(wv_c[cc][:], wvb[:])
                            else:
                                nc.scalar.activation(wv_c[cc][:], wvb[:], Copy)
                        for cc in range(CCH):
                            xb = xbfpool.tile([128, T], BF16, tag="xb")
                            eng = nc.sync if cc % 2 == 0 else nc.scalar
                            eng.dma_start(xb[:], xT[cc * 128 : (cc + 1) * 128, :])
                            if cc % 2 == 0:
                                nc.vector.tensor_copy(xT_c[cc][:], xb[:])
                            else:
                                nc.scalar.activation(xT_c[cc][:], xb[:], Copy)
                        # cf (bf16) after the x chunks on the SP queue
                        nc.sync.dma_start(cfq_sb[:D, :], cfq)
                        nc.sync.dma_start(cfk_sb[:D, :], cfk)
                        # v-proj sweep 1: 8 tk tiles accumulate in parallel,
                        # chunk-major, so PE has 8 matmuls of work per arriving
                        # x chunk instead of stalling on the full contraction.
                        NSW = 8
                        pvs = [
                            vps.tile([128, DG], F32, name=f"pv{tk}", tag=f"pv{tk}")
                            for tk in range(NSW)
                        ]
                        for cc in range(CCH):
                            for tk in range(NSW):
                                nc.tensor.matmul(
                                    pvs[tk][:],
                                    xT_c[cc][:, tk * 128 : (tk + 1) * 128],
                                    wv_c[cc][:],
                                    start=(cc == 0),
                                    stop=(cc == CCH - 1),
                                )
                        for tk in range(NSW):
                            vt = vstpool.tile([128, DG], BF16, tag="vt")
                            nc.vector.tensor_copy(vt[:], pvs[tk][:])
                            nc.sync.dma_start(v_s[tk * 128 : (tk + 1) * 128, :], vt[:])
                        # sweep 2: remaining tk at full speed (x resident)
                        for tk in range(NSW, NTK):
                            pv = vps.tile([128, DG], F32, name=f"pv{(tk - NSW) % NSW}", tag=f"pv{(tk - NSW) % NSW}")
                            for cc in range(CCH):
                                nc.tensor.matmul(
                                    pv[:],
                                    xT_c[cc][:, tk * 128 : (tk + 1) * 128],
                                    wv_c[cc][:],
                                    start=(cc == 0),
                                    stop=(cc == CCH - 1),
                                )
                            vt = vstpool.tile([128, DG], BF16, tag="vt")
                            nc.vector.tensor_copy(vt[:], pv[:])
                            nc.sync.dma_start(v_s[tk * 128 : (tk + 1) * 128, :], vt[:])

                    # ---- qT/kT ----
                    with (
                        tc.tile_pool(name="wbf", bufs=3) as wbfpool,
                        tc.tile_pool(name="wcc", bufs=2) as wccpool,
                        tc.tile_pool(name="qst", bufs=2) as qstpool,
                        tc.tile_pool(name="qkps", bufs=4, space="PSUM") as qkps,
                    ):
                        for h in range(HG):
                            for wi, (w_in, cf_sb) in enumerate(
                                ((wq, cfq_sb), (wk, cfk_sb))
                            ):
                                wb = wbfpool.tile([128, CCH, D], BF16, tag="wb")
                                for cc in range(CCH):
                                    nc.scalar.dma_start(
                                        wb[:, cc, :],
                                        w_in[
                                            cc * 128 : (cc + 1) * 128,
                                            h * D : (h + 1) * D,
                                        ],
                                    )
                                wc = wccpool.tile([128, CCH, D], F32R, tag="wc")
                                nc.vector.tensor_copy(wc[:], wb[:])
                                for tq in range(NTQ):
                                    pm = qkps.tile([128, TQC], F32)
                                    for cc in range(CCH):
                                        nc.tensor.matmul(
                                            pm[:],
                                            wc[:, cc, :],
                                            xT_c[cc][:, tq * TQC : (tq + 1) * TQC],
                                            start=(cc == 0),
                                            stop=(cc == CCH - 1),
                                        )
                                    if wi == 0:
                                        qs = qstpool.tile([128, TQC], BF16, tag="qs")
                                        nc.vector.tensor_mul(
                                            qs[:D, :],
                                            pm[:D, :],
                                            cf_sb[:D, tq * TQC : (tq + 1) * TQC],
                                        )
                                        nc.sync.dma_start(
                                            qT_s[
                                                h * D : (h + 1) * D,
                                                tq * TQC : (tq + 1) * TQC,
                                            ],
                                            qs[:D, :],
                                        )
                                    else:
                                        nc.vector.tensor_mul(
                                            kT_bf[:D, h, tq * TQC : (tq + 1) * TQC],
                                            pm[:D, :],
                                            cf_sb[:D, tq * TQC : (tq + 1) * TQC],
                                        )

            # ---------------- Phase B: attention ----------------
            with tc.tile_pool(name="o2", bufs=1) as o2pool:
                out2_sb = o2pool.tile([128, HG, T], F32R)
                with (
                    tc.tile_pool(name="wobf", bufs=2) as wobfpool,
                    tc.tile_pool(name="wop", bufs=1) as wopool,
                ):
                    with (
                        tc.tile_pool(name="hp", bufs=2) as hpool,
                        tc.tile_pool(name="hbf", bufs=2) as hbfpool,
                        tc.tile_pool(name="cst", bufs=1) as cstpool,
                        tc.tile_pool(name="ep", bufs=6) as epool,
                        tc.tile_pool(name="mp", bufs=4) as mpool,
                        tc.tile_pool(name="rp", bufs=2) as rpool,
                        tc.tile_pool(name="scps", bufs=4, space="PSUM") as scps,
                        tc.tile_pool(name="o2ps", bufs=2, space="PSUM") as o2ps,
                        tc.tile_pool(name="sps", bufs=2, space="PSUM") as sps,
                    ):
                        ones_f = cstpool.tile([128, 128], F32)
                        nc.vector.memset(ones_f[:], 1.0)
                        ones = cstpool.tile([128, 128], F32R)
                        nc.vector.tensor_copy(ones[:], ones_f[:])
                        # wo prefetch on the idle Pool queue
                        wo_sb = wopool.tile([128, HG, C], F32R)
                        for h in range(HG):
                            wob = wobfpool.tile([128, C], BF16, tag="wob")
                            nc.gpsimd.dma_start(
                                wob[:D, :], wo[h * D : (h + 1) * D, :]
                            )
                            if h % 2 == 0:
                                nc.vector.tensor_copy(wo_sb[:D, h, :], wob[:D, :])
                            else:
                                nc.scalar.activation(wo_sb[:D, h, :], wob[:D, :], Copy)
                        for h in range(HG):
                            qT_sb = hpool.tile([128, T], F32R, tag="qT")
                            kT_sb = hpool.tile([128, T], F32R, tag="kT")
                            v_sb = hpool.tile([128, NTK, D], F32R, tag="v")
                            qbf = hbfpool.tile([128, T], BF16, tag="qbf")
                            vbf = hbfpool.tile([128, NTK, D], BF16, tag="vbf")
                            nc.gpsimd.dma_start(qbf[:D, :], qT_s[h * D : (h + 1) * D, :])
                            for i in range(NTK):
                                nc.gpsimd.dma_start(
                                    vbf[:, i, :],
                                    v_s[i * 128 : (i + 1) * 128, h * D : (h + 1) * D],
                                )
                            nc.vector.tensor_copy(qT_sb[:D, :], qbf[:D, :])
                            nc.vector.tensor_copy(kT_sb[:D, :], kT_bf[:D, h, :])
                            nc.vector.tensor_copy(v_sb[:, :, :], vbf[:, :, :])
                            for tq in range(NTQ):
                                o2p = o2ps.tile([128, TQC], F32)
                                sp = sps.tile([128, TQC], F32)
                                pending = []
                                for tk in range(NTK):
                                    scp = scps.tile([128, TQC], F32)
                                    nc.tensor.matmul(
                                        scp[:],
                                        kT_sb[:D, tk * 128 : (tk + 1) * 128],
                                        qT_sb[:D, tq * TQC : (tq + 1) * TQC],
                                        start=True,
                                        stop=True,
                                    )
                                    et = epool.tile([128, TQC], F32R, tag="et")
                                    if use_mask:
                                        mt = mpool.tile([128, TQC], F32, tag="mt")
                                        nc.gpsimd.dma_start(
                                            mt[:],
                                            maskT[
                                                tk * 128 : (tk + 1) * 128,
                                                tq * TQC : (tq + 1) * TQC,
                                            ],
                                        )
                                        ma = mpool.tile([128, TQC], F32, tag="ma")
                                        nc.vector.tensor_add(ma[:], scp[:], mt[:])
                                        nc.scalar.activation(et[:], ma[:], Exp)
                                    else:
                                        nc.scalar.activation(et[:], scp[:], Exp)
                                    pending.append((et, tk))
                                    if len(pending) > 2:
                                        p_et, p_tk = pending.pop(0)
                                        nc.tensor.matmul(
                                            o2p[:],
                                            v_sb[:, p_tk, :],
                                            p_et[:],
                                            start=(p_tk == 0),
                                            stop=False,
                                        )
                                        nc.tensor.matmul(
                                            sp[:],
                                            ones[:],
                                            p_et[:],
                                            start=(p_tk == 0),
                                            stop=False,
                                        )
                                while pending:
                                    p_et, p_tk = pending.pop(0)
                                    nc.tensor.matmul(
                                        o2p[:],
                                        v_sb[:, p_tk, :],
                                        p_et[:],
                                        start=(p_tk == 0),
                                        stop=(p_tk == NTK - 1),
                                    )
                                    nc.tensor.matmul(
                                        sp[:],
                                        ones[:],
                                        p_et[:],
                                        start=(p_tk == 0),
                                        stop=(p_tk == NTK - 1),
                                    )
                                rt = rpool.tile([128, TQC], F32)
                                nc.vector.reciprocal(rt[:], sp[:])
                                nc.vector.tensor_mul(
                                    out2_sb[:D, h, tq * TQC : (tq + 1) * TQC],
                                    o2p[:D, :],
                                    rt[:D, :],
                                )

                    # -------- output projection --------
                    with (
                        tc.tile_pool(name="fst", bufs=4) as fpool,
                        tc.tile_pool(name="fps", bufs=4, space="PSUM") as fps,
                    ):
                        for qt in range(NQT):
                            for oc in range(NOC):
                                fp = fps.tile([128, TQC], F32)
                                for h in range(HG):
                                    nc.tensor.matmul(
                                        fp[:],
                                        out2_sb[:D, h, qt * 128 : (qt + 1) * 128],
                                        wo_sb[:D, h, oc * TQC : (oc + 1) * TQC],
                                        start=(h == 0),
                                        stop=(h == HG - 1),
                                    )
                                ft = fpool.tile([128, TQC], BF16, tag="ft")
                                if oc % 2 == 0:
                                    nc.vector.tensor_copy(ft[:], fp[:])
                                else:
                                    nc.scalar.activation(ft[:], fp[:], Copy)
                                nc.sync.dma_start(
                                    out[
                                        qt * 128 : (qt + 1) * 128,
                                        oc * TQC : (oc + 1) * TQC,
                                    ],
                                    ft[:],
                                )

    nc.compile()
    return nc


def compute_cfacs(T, D, theta=THETA):
    freq = 1.0 / theta ** (np.arange(0, D, 2, dtype=np.float64) / D)
    t = np.arange(T, dtype=np.float64)
    m = np.einsum("i,j->ij", t, freq)
    m = np.concatenate([m, m], axis=-1)
    cfac = (np.cos(m) + np.sin(m)).astype(np.float32)
    cfk = np.ascontiguousarray(cfac.T).astype(NP_BF16)
    cfq = np.ascontiguousarray(cfac.T / np.float32(math.sqrt(T))).astype(NP_BF16)
    return cfq, cfk


_NC_CACHE = {}


def _get_nc(use_mask):
    key = bool(use_mask)
    if key not in _NC_CACHE:
        _NC_CACHE[key] = build_attention_nc(SEQ, HIDDEN, HG, HEAD_DIM, use_mask=key)
    return _NC_CACHE[key]


def _make_in_maps(input_ids, Wq, Wk, Wv, Wo, attention_mask=None):
    DG = HG * HEAD_DIM
    cfq, cfk = compute_cfacs(SEQ, HEAD_DIM)
    xb = [np.ascontiguousarray(input_ids[bi].T).astype(NP_BF16) for bi in range(BATCH)]
    wqb = Wq.astype(NP_BF16)
    wkb = Wk.astype(NP_BF16)
    wvb = Wv.astype(NP_BF16)
    wob = Wo.astype(NP_BF16)
    in_maps = []
    for core in range(N_CORES):
        bi, g = divmod(core, MP)
        m = {
            "xT": xb[bi],
            "wq": np.ascontiguousarray(wqb[:, g * DG : (g + 1) * DG]),
            "wk": np.ascontiguousarray(wkb[:, g * DG : (g + 1) * DG]),
            "wv": np.ascontiguousarray(wvb[:, g * DG : (g + 1) * DG]),
            "wo": np.ascontiguousarray(wob[g * DG : (g + 1) * DG, :]),
            "cfq": cfq,
            "cfk": cfk,
        }
        if attention_mask is not None:
            m["maskT"] = np.ascontiguousarray(attention_mask[bi, 0].T)
        in_maps.append(m)
    return in_maps


def prepare_for_bench(inputs):
    input_ids = np.asarray(inputs["input_ids"], dtype=np.float32)
    Wq = np.asarray(inputs["Wq"], dtype=np.float32)
    Wk = np.asarray(inputs["Wk"], dtype=np.float32)
    Wv = np.asarray(inputs["Wv"], dtype=np.float32)
    Wo = np.asarray(inputs["Wo"], dtype=np.float32)
    return _get_nc(False), _make_in_maps(input_ids, Wq, Wk, Wv, Wo)


def kernel(input_ids, attention_mask, Wq, Wk, Wv, Wo):
    input_ids = np.asarray(input_ids, dtype=np.float32)
    attention_mask = np.asarray(attention_mask, dtype=np.float32)
    Wq = np.asarray(Wq, dtype=np.float32)
    Wk = np.asarray(Wk, dtype=np.float32)
    Wv = np.asarray(Wv, dtype=np.float32)
    Wo = np.asarray(Wo, dtype=np.float32)

    b, t, c = input_ids.shape
    assert (b, t, c) == (BATCH, SEQ, HIDDEN)

    use_mask = bool(np.any(attention_mask))
    nc = _get_nc(use_mask)
    in_maps = _make_in_maps(
        input_ids, Wq, Wk, Wv, Wo, attention_mask if use_mask else None
    )

    res = bass_utils.run_bass_kernel_spmd(nc, in_maps, core_ids=list(range(N_CORES)))

    out = np.zeros((BATCH, SEQ, HIDDEN), dtype=np.float32)
    for bi in range(BATCH):
        acc = res.results[bi * MP]["out"].astype(np.float32)
        for g in range(1, MP):
            acc = acc + res.results[bi * MP + g]["out"].astype(np.float32)
        out[bi] = acc
    return out
